# revision 1
# baseline (speedup 1.0000x reference)
"""Trainium2 Bass kernel for a single-head transformer block.

Reference computation (B=4, S=4096, D=1024, fp32):
    h   = rmsnorm(x) * g
    qkv = h @ w_qkv + b_qkv ;  q,k,v = split(qkv)
    q,k = ternary_rope(q), ternary_rope(k)      (cos/sin rounded to {-1,0,1})
    p   = softmax(q@k.T / sqrt(D) * ln3)        (base-3 softmax)
    out = (p @ v) @ w_proj + b_proj + x

Sharding: 8 cores, 2 per batch. Each core computes K/V for its full batch
(4096 keys) and attention for its 2048 query rows. Per-core inputs are
reordered so the core's own query rows come first (attention over keys is
permutation invariant); rope tables are passed per-core in the same order.

On-chip layout: activations are kept transposed (d on partitions) so that
scores land as S^T[key, q] and P @ V needs no transposes at all. All
matmuls run in bf16 (fp32 PSUM accumulate); the residual path stays fp32.
"""

import os
import numpy as np
import ml_dtypes

import concourse.bass as bass
import concourse.tile as tile
from concourse import mybir
from concourse.bass_utils import run_bass_kernel_spmd
from concourse.masks import make_identity

BF16 = mybir.dt.bfloat16
F32 = mybir.dt.float32

B, S, D = 4, 4096, 1024
P = 128
HALF = S // 2          # 2048 query rows per core
N_CORES = 8
RCH = 512              # row chunk for the qkv phase
N_RCH = S // RCH       # 8
N_QCH = HALF // RCH    # 4
NKT = S // P           # 32 key tiles
ND = D // P            # 8 d-slabs

EPS = 1e-6
LN3 = 1.0986122886681098
ROPE_BASE = 10000.0

LAST_RESULT = None     # BassKernelResults of the most recent run (for test.py)


def _split_multiwait(nc, max_waits=1):
    """Walrus in this build rejects instructions carrying many sem waits
    (the Tile end-of-kernel drain has one per engine/queue). Hoist excess
    waits onto single-wait NoOps just before the offending instruction."""
    for fn in nc.m.functions:
        for blk in fn.blocks:
            insts = list(blk.instructions)
            out, changed = [], False
            for ins in insts:
                si = ins.sync_info
                waits = list(si.on_wait) if si is not None and si.on_wait else []
                if len(waits) > max_waits:
                    changed = True
                    for j, w in enumerate(waits[:-max_waits]):
                        out.append(mybir.InstNoOp(
                            name=f"{ins.name}-sw{j}",
                            engine=ins.engine,
                            sync_info=mybir.SyncInfo(on_wait=[w], on_update=[]),
                            bass_nofuse=True,
                        ))
                    ins.sync_info = mybir.SyncInfo(
                        on_wait=waits[-max_waits:],
                        on_update=list(si.on_update) if si.on_update else [])
                out.append(ins)
            if changed:
                blk.instructions = out


def _ternary_tables(S=S):
    """Ternary rope cos/sin half-tables, transposed: [D/2, S] float32."""
    half = D // 2
    inv_freq = (1.0 / (ROPE_BASE ** (np.arange(half, dtype=np.float32) / half))
                ).astype(np.float32)
    ang = np.arange(S, dtype=np.float32)[:, None] * inv_freq[None, :]  # [S, half]
    cos = np.round(np.cos(ang)).astype(np.float32)
    sin = np.round(np.sin(ang)).astype(np.float32)
    return cos.T.copy(), sin.T.copy()  # [half, S]


def _prepare_in_maps(x, g_norm, w_qkv, b_qkv, w_proj, b_proj, S=S):
    HALF = S // 2
    cos_h, sin_h = _ternary_tables(S)
    wqkv_bf = np.ascontiguousarray(
        (g_norm[:, None] * w_qkv)).astype(ml_dtypes.bfloat16)
    wp_bf = np.ascontiguousarray(w_proj).astype(ml_dtypes.bfloat16)
    in_maps = []
    for c in range(N_CORES):
        b, h = c // 2, c % 2
        own = slice(h * HALF, (h + 1) * HALF)
        other = slice((1 - h) * HALF, (2 - h) * HALF)
        perm = np.concatenate([np.arange(own.start, own.stop),
                               np.arange(other.start, other.stop)])
        xb = x[b]
        in_maps.append({
            "x_t": np.ascontiguousarray(xb[perm]).astype(ml_dtypes.bfloat16),
            "res": np.ascontiguousarray(xb[own] + b_proj[None, :]),
            "wqkv": wqkv_bf,
            "wp": wp_bf,
            "bqkv": b_qkv,
            "cos_t": np.ascontiguousarray(cos_h[:, perm]).astype(ml_dtypes.bfloat16),
            "sin_t": np.ascontiguousarray(sin_h[:, perm]).astype(ml_dtypes.bfloat16),
        })
    return in_maps


def _build(has_bqkv: bool, S=S, ph12=True, ph3=True, split=True):
    HALF = S // 2
    N_RCH = S // RCH
    N_QCH = max(HALF // RCH, 1)
    NKT = S // P
    nc = bass.Bass("TRN2", target_bir_lowering=False, debug=False,
                   num_devices=N_CORES)

    x_t = nc.dram_tensor("x_t", [S, D], BF16, kind="ExternalInput").ap()
    res_d = nc.dram_tensor("res", [HALF, D], F32, kind="ExternalInput").ap()
    wqkv_d = nc.dram_tensor("wqkv", [D, 3 * D], BF16, kind="ExternalInput").ap()
    wp_d = nc.dram_tensor("wp", [D, D], BF16, kind="ExternalInput").ap()
    bqkv_d = nc.dram_tensor("bqkv", [3 * D], F32, kind="ExternalInput").ap()
    cos_d = nc.dram_tensor("cos_t", [D // 2, S], BF16, kind="ExternalInput").ap()
    sin_d = nc.dram_tensor("sin_t", [D // 2, S], BF16, kind="ExternalInput").ap()
    out_d = nc.dram_tensor("out", [HALF, D], F32, kind="ExternalOutput").ap()

    wqkv_r = wqkv_d.rearrange("(o p) n -> p o n", p=P)     # [128, 8, 3072]
    wp_r = wp_d.rearrange("(o p) n -> p o n", p=P)         # [128, 8, 1024]
    bqkv_r = bqkv_d.rearrange("(o p) -> p o", p=P)         # [128, 24]
    cos_r = cos_d.rearrange("(o p) s -> p o s", p=P)       # [128, 4, 4096]
    sin_r = sin_d.rearrange("(o p) s -> p o s", p=P)

    with tile.TileContext(nc) as tc:
        with (
            tc.tile_pool(name="singles", bufs=1) as singles,
            tc.tile_pool(name="dram", bufs=1, space="DRAM") as dram,
        ):
            ident = singles.tile([P, P], F32)
            make_identity(nc, ident)
            ones_bf = singles.tile([P, 1], BF16)
            nc.vector.memset(ones_bf, 1.0)
            onesc = singles.tile([1, P], BF16)
            nc.vector.memset(onesc, 1.0)
            eps_sb = singles.tile([P, 1], F32)
            nc.vector.memset(eps_sb, EPS)
            wp_sb = singles.tile([P, ND, D], BF16)
            nc.sync.dma_start(wp_sb, wp_r)
            bqkv_sb = singles.tile([P, 24], F32)
            nc.sync.dma_start(bqkv_sb, bqkv_r)

            kt_s = dram.tile([P, ND, S], BF16)      # rope'd K^T
            qt_s = dram.tile([P, ND, HALF], BF16)   # rope'd Q^T
            v_s = dram.tile([S, D], BF16)           # V, normal layout

            # ---------------- Phase 1+2: rmsnorm + QKV + rope ----------------
            if ph12:
                _phase12(nc, tc, S, has_bqkv, x_t, wqkv_r, cos_r, sin_r,
                         bqkv_d, bqkv_sb, ones_bf, onesc, eps_sb,
                         kt_s, qt_s, v_s)
            if ph3:
                _phase3(nc, tc, S, wp_sb, ident, res_d, out_d,
                        kt_s, qt_s, v_s)

    if split:
        _split_multiwait(nc)
    return nc


def _phase12(nc, tc, S, has_bqkv, x_t, wqkv_r, cos_r, sin_r, bqkv_d, bqkv_sb,
             ones_bf, onesc, eps_sb, kt_s, qt_s, v_s):
    N_RCH = S // RCH
    N_QCH = max((S // 2) // RCH, 1)
    if True:
            with (
                tc.tile_pool(name="wq", bufs=1) as wq_pool,
                tc.tile_pool(name="p12", bufs=2) as p12,
                tc.tile_pool(name="tmp12", bufs=3) as tmp12,
                tc.tile_pool(name="vout", bufs=3) as vout,
                tc.tile_pool(name="st", bufs=2) as st,
                tc.tile_pool(name="ps12", bufs=4, space="PSUM") as ps12,
                tc.tile_pool(name="psms", bufs=2, space="PSUM") as psms,
            ):
                wq_slabs = []
                for di in range(ND):
                    wsl = wq_pool.tile([P, 3 * D], BF16, tag=f"wq{di}",
                                       name=f"wq{di}")
                    nc.sync.dma_start(wsl, wqkv_r[:, di, :])
                    wq_slabs.append(wsl)

                for r in range(N_RCH):
                    rows = slice(r * RCH, (r + 1) * RCH)
                    # transpose-load x chunk: [512, 1024] -> x^T [128, 8, 512]
                    xT = p12.tile([P, ND, RCH], BF16, tag="xT")
                    for di in range(ND):
                        nc.sync.dma_start_transpose(
                            xT[:, di, :], x_t[rows, di * P:(di + 1) * P])
                    # mean(x^2) via PE partition-reduction of squares
                    sq = p12.tile([P, ND, RCH], BF16, tag="sq")
                    for di in range(ND):
                        nc.scalar.activation(sq[:, di, :], xT[:, di, :],
                                             mybir.ActivationFunctionType.Square)
                    ps_ms = psms.tile([1, RCH], F32, tag="ms")
                    for di in range(ND):
                        nc.tensor.matmul(ps_ms, ones_bf, sq[:, di, :],
                                         start=(di == 0), stop=(di == ND - 1))
                    sr = st.tile([1, RCH], F32, tag="sr")
                    nc.scalar.activation(sr, ps_ms,
                                         mybir.ActivationFunctionType.Sqrt,
                                         bias=eps_sb[0:1, :], scale=1.0 / D)
                    rf = st.tile([1, RCH], F32, tag="rf")
                    nc.vector.reciprocal(rf, sr)
                    rb = st.tile([1, RCH], BF16, tag="rb")
                    nc.vector.tensor_copy(rb, rf)
                    # broadcast r across partitions via a K=1 ones-matmul
                    # (tiny 1-partition DRAM-roundtrip DMAs fail NEFF load here)
                    psr = psms.tile([P, RCH], F32, tag="psr")
                    nc.tensor.matmul(psr, onesc, rb, start=True, stop=True)
                    rep = p12.tile([P, RCH], BF16, tag="rep")
                    nc.scalar.copy(rep, psr)
                    # h^T = x^T * r (broadcast over d)
                    hT = p12.tile([P, ND, RCH], BF16, tag="hT")
                    for di in range(ND):
                        nc.vector.tensor_tensor(hT[:, di, :], xT[:, di, :], rep,
                                                mybir.AluOpType.mult)

                    # rope tables for this chunk
                    cos_c = p12.tile([P, 4, RCH], BF16, tag="cos")
                    nc.sync.dma_start(cos_c, cos_r[:, :, rows])
                    sin_c = p12.tile([P, 4, RCH], BF16, tag="sin")
                    nc.sync.dma_start(sin_c, sin_r[:, :, rows])

                    # Q^T (chunks 0..3) and K^T: psum -> bf16 -> rope -> DRAM
                    sels = [("k", D, kt_s)] + ([("q", 0, qt_s)] if r < N_QCH else [])
                    for _, base, dst in sels:
                        t_qk = p12.tile([P, ND, RCH], BF16, tag="tqk")
                        for do in range(ND):
                            ps = ps12.tile([P, RCH], F32, tag="ps12")
                            for di in range(ND):
                                nc.tensor.matmul(
                                    ps,
                                    wq_slabs[di][:, base + do * P: base + (do + 1) * P],
                                    hT[:, di, :],
                                    start=(di == 0), stop=(di == ND - 1))
                            if has_bqkv:
                                nc.scalar.activation(
                                    t_qk[:, do, :], ps,
                                    mybir.ActivationFunctionType.Identity,
                                    bias=bqkv_sb[:, base // P + do: base // P + do + 1])
                            else:
                                nc.scalar.copy(t_qk[:, do, :], ps)
                        ro = p12.tile([P, ND, RCH], BF16, tag="ro")
                        for pr in range(4):
                            m1 = tmp12.tile([P, RCH], BF16, tag="m1")
                            nc.vector.tensor_tensor(m1, t_qk[:, pr, :],
                                                    cos_c[:, pr, :],
                                                    mybir.AluOpType.mult)
                            m2 = tmp12.tile([P, RCH], BF16, tag="m2")
                            nc.vector.tensor_tensor(m2, t_qk[:, pr + 4, :],
                                                    sin_c[:, pr, :],
                                                    mybir.AluOpType.mult)
                            nc.vector.tensor_tensor(ro[:, pr, :], m1, m2,
                                                    mybir.AluOpType.subtract)
                            m3 = tmp12.tile([P, RCH], BF16, tag="m3")
                            nc.vector.tensor_tensor(m3, t_qk[:, pr + 4, :],
                                                    cos_c[:, pr, :],
                                                    mybir.AluOpType.mult)
                            m4 = tmp12.tile([P, RCH], BF16, tag="m4")
                            nc.vector.tensor_tensor(m4, t_qk[:, pr, :],
                                                    sin_c[:, pr, :],
                                                    mybir.AluOpType.mult)
                            nc.vector.tensor_tensor(ro[:, pr + 4, :], m3, m4,
                                                    mybir.AluOpType.add)
                        for do in range(ND):
                            nc.sync.dma_start(dst[:, do, rows], ro[:, do, :])

                    # V (normal layout): lhsT = h^T row-block, rhs = Wv
                    for sub in range(RCH // P):
                        for no in range(D // 512):
                            ps = ps12.tile([P, RCH], F32, tag="ps12")
                            for di in range(ND):
                                nc.tensor.matmul(
                                    ps,
                                    hT[:, di, sub * P:(sub + 1) * P],
                                    wq_slabs[di][:, 2 * D + no * 512: 2 * D + (no + 1) * 512],
                                    start=(di == 0), stop=(di == ND - 1))
                            vt = vout.tile([P, 512], BF16, tag="vt")
                            if has_bqkv:
                                nc.scalar.copy(vt, ps)
                                nc.vector.tensor_tensor(
                                    vt, vt,
                                    bass.AP(tensor=bqkv_d.tensor,
                                            offset=bqkv_d.offset + 2 * D + no * 512,
                                            ap=[[0, P], [1, 512]]),
                                    mybir.AluOpType.add)
                            else:
                                nc.scalar.copy(vt, ps)
                            nc.sync.dma_start(
                                v_s[r * RCH + sub * P: r * RCH + (sub + 1) * P,
                                    no * 512:(no + 1) * 512], vt)

def _phase3(nc, tc, S, wp_sb, ident, res_d, out_d, kt_s, qt_s, v_s):
    N_QCH = max((S // 2) // RCH, 1)
    NKT = S // P
    NSUB = RCH // P
    if True:
            # ---------------- Phase 3: attention + proj + residual -----------
            with (
                tc.tile_pool(name="p3", bufs=2) as p3,
                tc.tile_pool(name="ktt", bufs=6) as kttp,
                tc.tile_pool(name="vst", bufs=4) as vstp,
                tc.tile_pool(name="outp", bufs=4) as outp,
                tc.tile_pool(name="rcp", bufs=4) as rcp,
                tc.tile_pool(name="ps_s", bufs=2, space="PSUM") as ps_s,
                tc.tile_pool(name="ps_pv", bufs=1, space="PSUM") as ps_pv,
                tc.tile_pool(name="ps_pj", bufs=2, space="PSUM") as ps_pj,
            ):
                for c in range(N_QCH):
                    qt = p3.tile([P, ND, RCH], BF16, tag="qt")
                    nc.sync.dma_start(qt, qt_s[:, :, c * RCH:(c + 1) * RCH])
                    pt = p3.tile([P, NKT, RCH], BF16, tag="pt")
                    acc = p3.tile([P, RCH], F32, tag="acc")
                    recip = rcp.tile([P, NSUB], F32, tag="recip")
                    if True:
                        for kt in range(NKT):
                            ktt = kttp.tile([P, ND, P], BF16, tag="ktt")
                            nc.sync.dma_start(ktt, kt_s[:, :, kt * P:(kt + 1) * P])
                            ps = ps_s.tile([P, RCH], F32, tag="ps_s")
                            for di in range(ND):
                                nc.tensor.matmul(ps, ktt[:, di, :], qt[:, di, :],
                                                 start=(di == 0), stop=(di == ND - 1))
                            nc.scalar.activation(pt[:, kt, :], ps,
                                                 mybir.ActivationFunctionType.Exp,
                                                 scale=LN3 / 32.0)
                            if kt == 0:
                                nc.vector.tensor_copy(acc, pt[:, 0, :])
                            else:
                                nc.vector.tensor_tensor(acc, acc, pt[:, kt, :],
                                                        mybir.AluOpType.add)
                        for i in range(NSUB):
                            pst = ps_s.tile([P, P], F32, tag="ps_s",
                                            name=f"pstr{c}_{i}")
                            nc.tensor.transpose(pst, acc[:, i * P:(i + 1) * P], ident)
                            scol = rcp.tile([P, 1], F32, tag="scol")
                            nc.vector.reduce_sum(scol, pst, axis=mybir.AxisListType.X)
                            nc.vector.reciprocal(recip[:, i:i + 1], scol)

                    ot = p3.tile([P, ND, RCH], BF16, tag="ot")
                    for g in range(2):
                        pvs = [ps_pv.tile([P, RCH], F32, tag=f"pv{j}",
                                          name=f"pv{c}_{g}_{j}")
                               for j in range(4)]
                        for kt in range(NKT):
                            vt = vstp.tile([P, 512], BF16, tag="vst",
                                           name=f"vt{c}_{g}_{kt}")
                            nc.sync.dma_start(
                                vt, v_s[kt * P:(kt + 1) * P,
                                        g * 512:(g + 1) * 512])
                            for j in range(4):
                                nc.tensor.matmul(pvs[j],
                                                 vt[:, j * P:(j + 1) * P],
                                                 pt[:, kt, :],
                                                 start=(kt == 0), stop=(kt == NKT - 1))
                        for j in range(4):
                            nc.scalar.copy(ot[:, g * 4 + j, :], pvs[j])

                    if True:
                        for qs in range(NSUB):
                            for no in range(D // 512):
                                ps = ps_pj.tile([P, 512], F32, tag="pj")
                                for di in range(ND):
                                    nc.tensor.matmul(
                                        ps, ot[:, di, qs * P:(qs + 1) * P],
                                        wp_sb[:, di, no * 512:(no + 1) * 512],
                                        start=(di == 0), stop=(di == ND - 1))
                                o1 = outp.tile([P, 512], F32, tag="o1")
                                nc.vector.tensor_scalar_mul(o1, ps,
                                                            recip[:, qs:qs + 1])
                                rt = outp.tile([P, 512], F32, tag="rt")
                                row0 = c * RCH + qs * P
                                nc.sync.dma_start(
                                    rt, res_d[row0:row0 + P, no * 512:(no + 1) * 512])
                                o2 = outp.tile([P, 512], F32, tag="o2")
                                nc.vector.tensor_tensor(o2, o1, rt,
                                                        mybir.AluOpType.add)
                                nc.sync.dma_start(
                                    out_d[row0:row0 + P, no * 512:(no + 1) * 512], o2)


_CACHED = {}


def kernel(x, g_norm, w_qkv, b_qkv, w_proj, b_proj):
    global LAST_RESULT
    x = np.asarray(x, dtype=np.float32)
    g_norm = np.asarray(g_norm, dtype=np.float32)
    w_qkv = np.asarray(w_qkv, dtype=np.float32)
    b_qkv = np.asarray(b_qkv, dtype=np.float32)
    w_proj = np.asarray(w_proj, dtype=np.float32)
    b_proj = np.asarray(b_proj, dtype=np.float32)

    has_bqkv = bool(np.any(b_qkv))
    key = ("nc", has_bqkv)
    if key not in _CACHED:
        _CACHED[key] = _build(has_bqkv)
    nc = _CACHED[key]

    in_maps = _prepare_in_maps(x, g_norm, w_qkv, b_qkv, w_proj, b_proj)
    LAST_RESULT = run_bass_kernel_spmd(nc, in_maps, list(range(N_CORES)),
                                       trace=False)
    out = np.empty((B, S, D), dtype=np.float32)
    for c in range(N_CORES):
        b, h = c // 2, c % 2
        out[b, h * HALF:(h + 1) * HALF, :] = LAST_RESULT.results[c]["out"]
    return out



# revision 3
# speedup vs baseline: 1.8983x; 1.8983x over previous
"""Trainium2 Bass kernel for a single-head transformer block.

Reference computation (B=4, S=4096, D=1024, fp32):
    h   = rmsnorm(x) * g
    qkv = h @ w_qkv + b_qkv ;  q,k,v = split(qkv)
    q,k = ternary_rope(q), ternary_rope(k)      (cos/sin rounded to {-1,0,1})
    p   = softmax(q@k.T / sqrt(D) * ln3)        (base-3 softmax)
    out = (p @ v) @ w_proj + b_proj + x

Sharding: 8 cores, 2 per batch. Each core computes K/V for its full batch
(4096 keys) and attention for its 2048 query rows. Per-core inputs are
reordered so the core's own query rows come first (attention over keys is
permutation invariant); rope tables are passed per-core in the same order.

All heavy matmuls run in fp8 e4m3 with DoubleRow perf mode (K=256 per
instruction, 2x PE throughput). The attention path contributes ~1% of the
output norm (the fp32 residual dominates), so fp8 keeps rel err ~7e-4.
K^T, Q^T and V live in SBUF for the whole kernel - no DRAM roundtrips.
The unnormalized attention output is scaled by 1/64 before fp8 quantization
(folded back via the softmax-sum reciprocal, whose transpose uses a 1/64
pre-scaled identity).
"""

import numpy as np
import ml_dtypes

import concourse.bass as bass
import concourse.tile as tile
from concourse import mybir
from concourse.bass_utils import run_bass_kernel_spmd
from concourse.masks import make_identity

F8 = mybir.dt.float8e4
BF16 = mybir.dt.bfloat16
F32 = mybir.dt.float32
NP_F8 = ml_dtypes.float8_e4m3

B, S, D = 4, 4096, 1024
P = 128
HALF = S // 2          # 2048 query rows per core
N_CORES = 8
RCH = 512              # row chunk for the qkv phase
N_RCH = S // RCH       # 8
N_QCH = HALF // RCH    # 4
NKT = S // P           # 32 key tiles
ND = D // P            # 8 d-slabs
OSCALE = 1.0 / 64.0    # pre-quantization scale for unnormalized attn out

EPS = 1e-6
LN3 = 1.0986122886681098
ROPE_BASE = 10000.0

DR = mybir.MatmulPerfMode.DoubleRow

LAST_RESULT = None     # BassKernelResults of the most recent run (for test.py)


def _split_multiwait(nc, max_waits=1):
    """Walrus in this build rejects instructions carrying many sem waits
    (the Tile end-of-kernel drain has one per engine/queue). Hoist excess
    waits onto single-wait NoOps just before the offending instruction."""
    for fn in nc.m.functions:
        for blk in fn.blocks:
            insts = list(blk.instructions)
            out, changed = [], False
            for ins in insts:
                si = ins.sync_info
                waits = list(si.on_wait) if si is not None and si.on_wait else []
                if len(waits) > max_waits:
                    changed = True
                    for j, w in enumerate(waits[:-max_waits]):
                        out.append(mybir.InstNoOp(
                            name=f"{ins.name}-sw{j}",
                            engine=ins.engine,
                            sync_info=mybir.SyncInfo(on_wait=[w], on_update=[]),
                            bass_nofuse=True,
                        ))
                    ins.sync_info = mybir.SyncInfo(
                        on_wait=waits[-max_waits:],
                        on_update=list(si.on_update) if si.on_update else [])
                out.append(ins)
            if changed:
                blk.instructions = out


def _ternary_tables(S=S):
    """Ternary rope cos/sin half-tables, transposed: [D/2, S] float32."""
    half = D // 2
    inv_freq = (1.0 / (ROPE_BASE ** (np.arange(half, dtype=np.float32) / half))
                ).astype(np.float32)
    ang = np.arange(S, dtype=np.float32)[:, None] * inv_freq[None, :]  # [S, half]
    cos = np.round(np.cos(ang)).astype(np.float32)
    sin = np.round(np.sin(ang)).astype(np.float32)
    return cos.T.copy(), sin.T.copy()  # [half, S]


def _prepare_in_maps(x, g_norm, w_qkv, b_qkv, w_proj, b_proj, S=S):
    HALF = S // 2
    cos_h, sin_h = _ternary_tables(S)
    wqkv_f8 = np.ascontiguousarray(
        (g_norm[:, None] * w_qkv)).astype(NP_F8)
    wp_f8 = np.ascontiguousarray(w_proj).astype(NP_F8)
    in_maps = []
    for c in range(N_CORES):
        b, h = c // 2, c % 2
        own = slice(h * HALF, (h + 1) * HALF)
        other = slice((1 - h) * HALF, (2 - h) * HALF)
        perm = np.concatenate([np.arange(own.start, own.stop),
                               np.arange(other.start, other.stop)])
        xb = x[b]
        in_maps.append({
            # x^T, column-permuted so own rows come first: [D, S]
            "x_t": np.ascontiguousarray(xb[perm].T).astype(ml_dtypes.bfloat16),
            "res": np.ascontiguousarray(xb[own] + b_proj[None, :]),
            "wqkv": wqkv_f8,
            "wp": wp_f8,
            "bqkv": b_qkv,
            "cos_t": np.ascontiguousarray(cos_h[:, perm]).astype(ml_dtypes.bfloat16),
            "sin_t": np.ascontiguousarray(sin_h[:, perm]).astype(ml_dtypes.bfloat16),
        })
    return in_maps


def _build(has_bqkv: bool, S=S, ph12=True, ph3=True, split=True):
    HALF = S // 2
    N_RCH = S // RCH
    N_QCH = max(HALF // RCH, 1)
    nc = bass.Bass("TRN2", target_bir_lowering=False, debug=False,
                   num_devices=N_CORES)

    x_t = nc.dram_tensor("x_t", [D, S], BF16, kind="ExternalInput").ap()
    res_d = nc.dram_tensor("res", [HALF, D], F32, kind="ExternalInput").ap()
    wqkv_d = nc.dram_tensor("wqkv", [D, 3 * D], F8, kind="ExternalInput").ap()
    wp_d = nc.dram_tensor("wp", [D, D], F8, kind="ExternalInput").ap()
    bqkv_d = nc.dram_tensor("bqkv", [3 * D], F32, kind="ExternalInput").ap()
    cos_d = nc.dram_tensor("cos_t", [D // 2, S], BF16, kind="ExternalInput").ap()
    sin_d = nc.dram_tensor("sin_t", [D // 2, S], BF16, kind="ExternalInput").ap()
    out_d = nc.dram_tensor("out", [HALF, D], F32, kind="ExternalOutput").ap()

    x_r = x_t.rearrange("(o p) s -> p o s", p=P)           # [128, 8, 4096]
    wqkv_r = wqkv_d.rearrange("(o p) n -> p o n", p=P)     # [128, 8, 3072]
    wp_r = wp_d.rearrange("(o p) n -> p o n", p=P)         # [128, 8, 1024]
    bqkv_r = bqkv_d.rearrange("(o p) -> p o", p=P)         # [128, 24]
    cos_r = cos_d.rearrange("(o p) s -> p o s", p=P)       # [128, 4, 4096]
    sin_r = sin_d.rearrange("(o p) s -> p o s", p=P)

    with tile.TileContext(nc) as tc:
        with tc.tile_pool(name="singles", bufs=1) as singles:
            ident64 = singles.tile([P, P], F32)
            make_identity(nc, ident64)
            nc.vector.tensor_scalar_mul(ident64, ident64, OSCALE)
            ones8_pad = singles.tile([P, 2, 16], F8)
            nc.vector.memset(ones8_pad, 1.0)
            ones8 = ones8_pad[:, :, 0:1]
            onesc = singles.tile([1, P], BF16)
            nc.vector.memset(onesc, 1.0)
            eps_sb = singles.tile([1, 1], F32)
            nc.vector.memset(eps_sb, EPS)
            wqkv_sb = singles.tile([P, ND, 3 * D], F8)
            for o in range(ND):
                nc.sync.dma_start(wqkv_sb[:, o, :], wqkv_r[:, o, :])
            wp_sb = singles.tile([P, ND, D], F8)
            nc.sync.dma_start(wp_sb, wp_r)
            bqkv_sb = singles.tile([P, 24], F32)
            nc.sync.dma_start(bqkv_sb, bqkv_r)

            kt_s = singles.tile([P, ND, S], F8)       # rope'd K^T (SBUF-resident)
            qt_s = singles.tile([P, ND, HALF], F8)    # rope'd Q^T
            v_s = singles.tile([P, NKT, D], F8)       # V, keys on partitions

            if ph12:
                _phase12(nc, tc, S, has_bqkv, x_r, wqkv_sb, cos_r, sin_r,
                         bqkv_d, bqkv_sb, ones8, onesc, eps_sb,
                         kt_s, qt_s, v_s)
            if ph3:
                _phase3(nc, tc, S, wp_sb, ident64, res_d, out_d,
                        kt_s, qt_s, v_s)

    if split:
        _split_multiwait(nc)
    return nc


def _phase12(nc, tc, S, has_bqkv, x_r, wqkv_sb, cos_r, sin_r, bqkv_d, bqkv_sb,
             ones8, onesc, eps_sb, kt_s, qt_s, v_s):
    N_RCH = S // RCH
    N_QCH = max((S // 2) // RCH, 1)
    with (
        tc.tile_pool(name="p12", bufs=2) as p12,
        tc.tile_pool(name="tmp12", bufs=3) as tmp12,
        tc.tile_pool(name="st", bufs=2) as st,
        tc.tile_pool(name="ps12", bufs=4, space="PSUM") as ps12,
        tc.tile_pool(name="psms", bufs=2, space="PSUM") as psms,
    ):
        for r in range(N_RCH):
            rows = slice(r * RCH, (r + 1) * RCH)
            xT = p12.tile([P, ND, RCH], BF16, tag="xT")
            for o in range(ND):
                nc.sync.dma_start(xT[:, o, :], x_r[:, o, rows])
            # mean(x^2) via PE partition-reduction of squares (fp8 DoubleRow)
            sq = p12.tile([P, ND, RCH], F8, tag="sq")
            nc.scalar.activation(sq, xT, mybir.ActivationFunctionType.Square)
            ps_ms = psms.tile([1, RCH], F32, tag="ms")
            for i in range(ND // 2):
                nc.tensor.matmul(ps_ms, ones8, sq[:, 2 * i:2 * i + 2, :],
                                 start=(i == 0), stop=(i == ND // 2 - 1),
                                 perf_mode=DR)
            sr = st.tile([1, RCH], F32, tag="sr")
            nc.scalar.activation(sr, ps_ms,
                                 mybir.ActivationFunctionType.Sqrt,
                                 bias=eps_sb, scale=1.0 / D)
            rf = st.tile([1, RCH], F32, tag="rf")
            nc.vector.reciprocal(rf, sr)
            rb = st.tile([1, RCH], BF16, tag="rb")
            nc.vector.tensor_copy(rb, rf)
            # broadcast r across partitions via a K=1 ones-matmul
            psr = psms.tile([P, RCH], F32, tag="psr")
            nc.tensor.matmul(psr, onesc, rb, start=True, stop=True)
            rep = p12.tile([P, RCH], BF16, tag="rep")
            nc.scalar.copy(rep, psr)
            # h^T = x^T * r (broadcast over d), quantized to fp8
            hT = p12.tile([P, ND, RCH], F8, tag="hT")
            for di in range(ND):
                nc.vector.tensor_tensor(hT[:, di, :], xT[:, di, :], rep,
                                        mybir.AluOpType.mult)

            cos_c = p12.tile([P, 4, RCH], BF16, tag="cos")
            nc.sync.dma_start(cos_c, cos_r[:, :, rows])
            sin_c = p12.tile([P, 4, RCH], BF16, tag="sin")
            nc.sync.dma_start(sin_c, sin_r[:, :, rows])

            # Q^T (chunks 0..3) and K^T: psum -> bf16 -> rope -> fp8 resident
            sels = [("k", D, kt_s)] + ([("q", 0, qt_s)] if r < N_QCH else [])
            for _, base, dst in sels:
                t_qk = p12.tile([P, ND, RCH], BF16, tag="tqk")
                for do in range(ND):
                    ps = ps12.tile([P, RCH], F32, tag="ps12")
                    for i in range(ND // 2):
                        nc.tensor.matmul(
                            ps,
                            wqkv_sb[:, 2 * i:2 * i + 2,
                                    base + do * P: base + (do + 1) * P],
                            hT[:, 2 * i:2 * i + 2, :],
                            start=(i == 0), stop=(i == ND // 2 - 1),
                            perf_mode=DR)
                    if has_bqkv:
                        nc.scalar.activation(
                            t_qk[:, do, :], ps,
                            mybir.ActivationFunctionType.Identity,
                            bias=bqkv_sb[:, base // P + do: base // P + do + 1])
                    else:
                        nc.scalar.copy(t_qk[:, do, :], ps)
                # rope: pair block pr with pr+4, write fp8 into resident dst
                for pr in range(4):
                    m1 = tmp12.tile([P, RCH], BF16, tag="m1")
                    nc.vector.tensor_tensor(m1, t_qk[:, pr, :],
                                            cos_c[:, pr, :],
                                            mybir.AluOpType.mult)
                    m2 = tmp12.tile([P, RCH], BF16, tag="m2")
                    nc.vector.tensor_tensor(m2, t_qk[:, pr + 4, :],
                                            sin_c[:, pr, :],
                                            mybir.AluOpType.mult)
                    nc.vector.tensor_tensor(dst[:, pr, rows], m1, m2,
                                            mybir.AluOpType.subtract)
                    m3 = tmp12.tile([P, RCH], BF16, tag="m3")
                    nc.vector.tensor_tensor(m3, t_qk[:, pr + 4, :],
                                            cos_c[:, pr, :],
                                            mybir.AluOpType.mult)
                    m4 = tmp12.tile([P, RCH], BF16, tag="m4")
                    nc.vector.tensor_tensor(m4, t_qk[:, pr, :],
                                            sin_c[:, pr, :],
                                            mybir.AluOpType.mult)
                    nc.vector.tensor_tensor(dst[:, pr + 4, rows], m3, m4,
                                            mybir.AluOpType.add)

            # V (keys on partitions): lhsT = h^T row-block, rhs = Wv
            for sub in range(RCH // P):
                for no in range(D // 512):
                    ps = ps12.tile([P, RCH], F32, tag="ps12")
                    for i in range(ND // 2):
                        nc.tensor.matmul(
                            ps,
                            hT[:, 2 * i:2 * i + 2, sub * P:(sub + 1) * P],
                            wqkv_sb[:, 2 * i:2 * i + 2,
                                    2 * D + no * 512: 2 * D + (no + 1) * 512],
                            start=(i == 0), stop=(i == ND // 2 - 1),
                            perf_mode=DR)
                    vdst = v_s[:, r * (RCH // P) + sub, no * 512:(no + 1) * 512]
                    if has_bqkv:
                        vt = tmp12.tile([P, 512], BF16, tag="vtb")
                        nc.scalar.copy(vt, ps)
                        nc.vector.tensor_tensor(
                            vdst, vt,
                            bass.AP(tensor=bqkv_d.tensor,
                                    offset=bqkv_d.offset + 2 * D + no * 512,
                                    ap=[[0, P], [1, 512]]),
                            mybir.AluOpType.add)
                    else:
                        nc.scalar.copy(vdst, ps)


def _phase3(nc, tc, S, wp_sb, ident64, res_d, out_d, kt_s, qt_s, v_s):
    N_QCH = max((S // 2) // RCH, 1)
    NKT = S // P
    NSUB = RCH // P
    with (
        tc.tile_pool(name="p3", bufs=2) as p3,
        tc.tile_pool(name="outp", bufs=4) as outp,
        tc.tile_pool(name="rcp", bufs=4) as rcp,
        tc.tile_pool(name="ps_s", bufs=2, space="PSUM") as ps_s,
        tc.tile_pool(name="ps_pv", bufs=1, space="PSUM") as ps_pv,
        tc.tile_pool(name="ps_pj", bufs=2, space="PSUM") as ps_pj,
    ):
        for c in range(N_QCH):
            qcols = slice(c * RCH, (c + 1) * RCH)
            pt = p3.tile([P, NKT, RCH], F8, tag="pt")
            acc = p3.tile([P, RCH], F32, tag="acc")
            recip = rcp.tile([P, NSUB], F32, tag="recip")
            for kt in range(NKT):
                ps = ps_s.tile([P, RCH], F32, tag="ps_s")
                for i in range(ND // 2):
                    nc.tensor.matmul(ps,
                                     kt_s[:, 2 * i:2 * i + 2, kt * P:(kt + 1) * P],
                                     qt_s[:, 2 * i:2 * i + 2, qcols],
                                     start=(i == 0), stop=(i == ND // 2 - 1),
                                     perf_mode=DR)
                nc.scalar.activation(pt[:, kt, :], ps,
                                     mybir.ActivationFunctionType.Exp,
                                     scale=LN3 / 32.0)
                if kt == 0:
                    nc.vector.tensor_copy(acc, pt[:, 0, :])
                else:
                    nc.vector.tensor_tensor(acc, acc, pt[:, kt, :],
                                            mybir.AluOpType.add)
            # per-query softmax sum: transpose (pre-scaled by 1/64) + reduce
            for i in range(NSUB):
                pst = ps_s.tile([P, P], F32, tag="ps_s", name=f"pstr{c}_{i}")
                nc.tensor.transpose(pst, acc[:, i * P:(i + 1) * P], ident64)
                scol = rcp.tile([P, 1], F32, tag="scol")
                nc.vector.reduce_sum(scol, pst, axis=mybir.AxisListType.X)
                nc.vector.reciprocal(recip[:, i:i + 1], scol)

            # attn @ V, unnormalized, scaled by 1/64 into fp8
            ot = p3.tile([P, ND, RCH], F8, tag="ot")
            for g in range(2):
                pvs = [ps_pv.tile([P, RCH], F32, tag=f"pv{j}",
                                  name=f"pv{c}_{g}_{j}")
                       for j in range(4)]
                for t in range(NKT // 2):
                    for j in range(4):
                        nc.tensor.matmul(
                            pvs[j],
                            v_s[:, 2 * t:2 * t + 2,
                                g * 512 + j * P: g * 512 + (j + 1) * P],
                            pt[:, 2 * t:2 * t + 2, :],
                            start=(t == 0), stop=(t == NKT // 2 - 1),
                            perf_mode=DR)
                for j in range(4):
                    nc.scalar.activation(ot[:, g * 4 + j, :], pvs[j],
                                         mybir.ActivationFunctionType.Copy,
                                         scale=OSCALE)

            # out = (ot @ wp) * (64/sum) + res
            for qs in range(NSUB):
                for no in range(D // 512):
                    ps = ps_pj.tile([P, 512], F32, tag="pj")
                    for i in range(ND // 2):
                        nc.tensor.matmul(
                            ps, ot[:, 2 * i:2 * i + 2, qs * P:(qs + 1) * P],
                            wp_sb[:, 2 * i:2 * i + 2, no * 512:(no + 1) * 512],
                            start=(i == 0), stop=(i == ND // 2 - 1),
                            perf_mode=DR)
                    o1 = outp.tile([P, 512], F32, tag="o1")
                    nc.vector.tensor_scalar_mul(o1, ps, recip[:, qs:qs + 1])
                    rt = outp.tile([P, 512], F32, tag="rt")
                    row0 = c * RCH + qs * P
                    nc.sync.dma_start(
                        rt, res_d[row0:row0 + P, no * 512:(no + 1) * 512])
                    o2 = outp.tile([P, 512], F32, tag="o2")
                    nc.vector.tensor_tensor(o2, o1, rt,
                                            mybir.AluOpType.add)
                    nc.sync.dma_start(
                        out_d[row0:row0 + P, no * 512:(no + 1) * 512], o2)


_CACHED = {}


def kernel(x, g_norm, w_qkv, b_qkv, w_proj, b_proj):
    global LAST_RESULT
    x = np.asarray(x, dtype=np.float32)
    g_norm = np.asarray(g_norm, dtype=np.float32)
    w_qkv = np.asarray(w_qkv, dtype=np.float32)
    b_qkv = np.asarray(b_qkv, dtype=np.float32)
    w_proj = np.asarray(w_proj, dtype=np.float32)
    b_proj = np.asarray(b_proj, dtype=np.float32)

    has_bqkv = bool(np.any(b_qkv))
    key = ("nc", has_bqkv)
    if key not in _CACHED:
        _CACHED[key] = _build(has_bqkv)
    nc = _CACHED[key]

    in_maps = _prepare_in_maps(x, g_norm, w_qkv, b_qkv, w_proj, b_proj)
    LAST_RESULT = run_bass_kernel_spmd(nc, in_maps, list(range(N_CORES)),
                                       trace=False)
    out = np.empty((B, S, D), dtype=np.float32)
    for c in range(N_CORES):
        b, h = c // 2, c % 2
        out[b, h * HALF:(h + 1) * HALF, :] = LAST_RESULT.results[c]["out"]
    return out


# revision 4
# speedup vs baseline: 1.9300x; 1.0167x over previous
"""Trainium2 Bass kernel for a single-head transformer block.

Reference computation (B=4, S=4096, D=1024, fp32):
    h   = rmsnorm(x) * g
    qkv = h @ w_qkv + b_qkv ;  q,k,v = split(qkv)
    q,k = ternary_rope(q), ternary_rope(k)      (cos/sin rounded to {-1,0,1})
    p   = softmax(q@k.T / sqrt(D) * ln3)        (base-3 softmax)
    out = (p @ v) @ w_proj + b_proj + x

Sharding: 8 cores, 2 per batch. Each core computes K/V for its full batch
(4096 keys) and attention for its 2048 query rows. Per-core inputs are
reordered so the core's own query rows come first (attention over keys is
permutation invariant); rope tables are passed per-core in the same order.

All heavy matmuls run in fp8 e4m3 with DoubleRow perf mode (K=256 per
instruction, 2x PE throughput). The attention path contributes ~1% of the
output norm (the fp32 residual dominates), so fp8 keeps rel err ~7e-4.
K^T, Q^T and V live in SBUF for the whole kernel - no DRAM roundtrips.
The unnormalized attention output is scaled by 1/64 before fp8 quantization
(folded back via the softmax-sum reciprocal, whose transpose uses a 1/64
pre-scaled identity).
"""

import numpy as np
import ml_dtypes

import concourse.bass as bass
import concourse.tile as tile
from concourse import mybir
from concourse.bass_utils import run_bass_kernel_spmd
from concourse.masks import make_identity

F8 = mybir.dt.float8e4
BF16 = mybir.dt.bfloat16
F32 = mybir.dt.float32
NP_F8 = ml_dtypes.float8_e4m3

B, S, D = 4, 4096, 1024
P = 128
HALF = S // 2          # 2048 query rows per core
N_CORES = 8
RCH = 512              # row chunk for the qkv phase
N_RCH = S // RCH       # 8
N_QCH = HALF // RCH    # 4
NKT = S // P           # 32 key tiles
ND = D // P            # 8 d-slabs
OSCALE = 1.0 / 64.0    # pre-quantization scale for unnormalized attn out
WSCALE = 16.0          # fp8 weight pre-scale (keeps w out of the subnormal
                       # flush-to-zero range); undone in the psum copies

EPS = 1e-6
LN3 = 1.0986122886681098
ROPE_BASE = 10000.0

DR = mybir.MatmulPerfMode.DoubleRow

LAST_RESULT = None     # BassKernelResults of the most recent run (for test.py)


def _split_multiwait(nc, max_waits=1):
    """Walrus in this build rejects instructions carrying many sem waits
    (the Tile end-of-kernel drain has one per engine/queue). Hoist excess
    waits onto single-wait NoOps just before the offending instruction."""
    for fn in nc.m.functions:
        for blk in fn.blocks:
            insts = list(blk.instructions)
            out, changed = [], False
            for ins in insts:
                si = ins.sync_info
                waits = list(si.on_wait) if si is not None and si.on_wait else []
                if len(waits) > max_waits:
                    changed = True
                    for j, w in enumerate(waits[:-max_waits]):
                        out.append(mybir.InstNoOp(
                            name=f"{ins.name}-sw{j}",
                            engine=ins.engine,
                            sync_info=mybir.SyncInfo(on_wait=[w], on_update=[]),
                            bass_nofuse=True,
                        ))
                    ins.sync_info = mybir.SyncInfo(
                        on_wait=waits[-max_waits:],
                        on_update=list(si.on_update) if si.on_update else [])
                out.append(ins)
            if changed:
                blk.instructions = out


def _ternary_tables(S=S):
    """Ternary rope cos/sin half-tables, transposed: [D/2, S] float32."""
    half = D // 2
    inv_freq = (1.0 / (ROPE_BASE ** (np.arange(half, dtype=np.float32) / half))
                ).astype(np.float32)
    ang = np.arange(S, dtype=np.float32)[:, None] * inv_freq[None, :]  # [S, half]
    cos = np.round(np.cos(ang)).astype(np.float32)
    sin = np.round(np.sin(ang)).astype(np.float32)
    return cos.T.copy(), sin.T.copy()  # [half, S]


def _prepare_in_maps(x, g_norm, w_qkv, b_qkv, w_proj, b_proj, S=S):
    HALF = S // 2
    cos_h, sin_h = _ternary_tables(S)
    wqkv_f8 = np.ascontiguousarray(
        (g_norm[:, None] * w_qkv * WSCALE)).astype(NP_F8)
    wp_f8 = np.ascontiguousarray(w_proj * WSCALE).astype(NP_F8)
    in_maps = []
    for c in range(N_CORES):
        b, h = c // 2, c % 2
        own = slice(h * HALF, (h + 1) * HALF)
        other = slice((1 - h) * HALF, (2 - h) * HALF)
        perm = np.concatenate([np.arange(own.start, own.stop),
                               np.arange(other.start, other.stop)])
        xb = x[b]
        in_maps.append({
            # x^T, column-permuted so own rows come first: [D, S]
            "x_t": np.ascontiguousarray(xb[perm].T).astype(ml_dtypes.bfloat16),
            "res": np.ascontiguousarray(xb[own] + b_proj[None, :]),
            "wqkv": wqkv_f8,
            "wp": wp_f8,
            "bqkv": b_qkv,
            "cos_t": np.ascontiguousarray(cos_h[:, perm]).astype(ml_dtypes.bfloat16),
            "sin_t": np.ascontiguousarray(sin_h[:, perm]).astype(ml_dtypes.bfloat16),
        })
    return in_maps


def _build(has_bqkv: bool, S=S, ph12=True, ph3=True, split=True):
    HALF = S // 2
    N_RCH = S // RCH
    N_QCH = max(HALF // RCH, 1)
    nc = bass.Bass("TRN2", target_bir_lowering=False, debug=False,
                   num_devices=N_CORES)

    x_t = nc.dram_tensor("x_t", [D, S], BF16, kind="ExternalInput").ap()
    res_d = nc.dram_tensor("res", [HALF, D], F32, kind="ExternalInput").ap()
    wqkv_d = nc.dram_tensor("wqkv", [D, 3 * D], F8, kind="ExternalInput").ap()
    wp_d = nc.dram_tensor("wp", [D, D], F8, kind="ExternalInput").ap()
    bqkv_d = nc.dram_tensor("bqkv", [3 * D], F32, kind="ExternalInput").ap()
    cos_d = nc.dram_tensor("cos_t", [D // 2, S], BF16, kind="ExternalInput").ap()
    sin_d = nc.dram_tensor("sin_t", [D // 2, S], BF16, kind="ExternalInput").ap()
    out_d = nc.dram_tensor("out", [HALF, D], F32, kind="ExternalOutput").ap()

    x_r = x_t.rearrange("(o p) s -> p o s", p=P)           # [128, 8, 4096]
    wqkv_r = wqkv_d.rearrange("(o p) n -> p o n", p=P)     # [128, 8, 3072]
    wp_r = wp_d.rearrange("(o p) n -> p o n", p=P)         # [128, 8, 1024]
    bqkv_r = bqkv_d.rearrange("(o p) -> p o", p=P)         # [128, 24]
    cos_r = cos_d.rearrange("(o p) s -> p o s", p=P)       # [128, 4, 4096]
    sin_r = sin_d.rearrange("(o p) s -> p o s", p=P)

    with tile.TileContext(nc) as tc:
        with tc.tile_pool(name="singles", bufs=1) as singles:
            ident64 = singles.tile([P, P], F32)
            make_identity(nc, ident64)
            nc.vector.tensor_scalar_mul(ident64, ident64, OSCALE * WSCALE)
            ones8_pad = singles.tile([P, 2, 16], F8)
            nc.vector.memset(ones8_pad, 1.0)
            ones8 = ones8_pad[:, :, 0:1]
            onesc = singles.tile([1, P], BF16)
            nc.vector.memset(onesc, 1.0)
            eps_sb = singles.tile([1, 1], F32)
            nc.vector.memset(eps_sb, EPS)
            wqkv_sb = singles.tile([P, ND, 3 * D], F8)
            for o in range(ND):
                nc.sync.dma_start(wqkv_sb[:, o, :], wqkv_r[:, o, :])
            wp_sb = singles.tile([P, ND, D], F8)
            nc.sync.dma_start(wp_sb, wp_r)
            bqkv_sb = singles.tile([P, 24], F32)
            nc.sync.dma_start(bqkv_sb, bqkv_r)

            kt_s = singles.tile([P, ND, S], F8)       # rope'd K^T (SBUF-resident)
            qt_s = singles.tile([P, ND, HALF], F8)    # rope'd Q^T
            v_s = singles.tile([P, NKT, D], F8)       # V, keys on partitions

            if ph12:
                _phase12(nc, tc, S, has_bqkv, x_r, wqkv_sb, cos_r, sin_r,
                         bqkv_d, bqkv_sb, ones8, onesc, eps_sb,
                         kt_s, qt_s, v_s)
            if ph3:
                _phase3(nc, tc, S, wp_sb, ident64, res_d, out_d,
                        kt_s, qt_s, v_s)

    if split:
        _split_multiwait(nc)
    return nc


def _phase12(nc, tc, S, has_bqkv, x_r, wqkv_sb, cos_r, sin_r, bqkv_d, bqkv_sb,
             ones8, onesc, eps_sb, kt_s, qt_s, v_s):
    N_RCH = S // RCH
    N_QCH = max((S // 2) // RCH, 1)
    with (
        tc.tile_pool(name="p12", bufs=2) as p12,
        tc.tile_pool(name="tmp12", bufs=3) as tmp12,
        tc.tile_pool(name="st", bufs=2) as st,
        tc.tile_pool(name="ps12", bufs=4, space="PSUM") as ps12,
        tc.tile_pool(name="psms", bufs=2, space="PSUM") as psms,
    ):
        for r in range(N_RCH):
            rows = slice(r * RCH, (r + 1) * RCH)
            xT = p12.tile([P, ND, RCH], BF16, tag="xT")
            for o in range(ND):
                nc.sync.dma_start(xT[:, o, :], x_r[:, o, rows])
            # mean(x^2) via PE partition-reduction of squares (fp8 DoubleRow)
            sq = p12.tile([P, ND, RCH], F8, tag="sq")
            nc.scalar.activation(sq, xT, mybir.ActivationFunctionType.Square)
            ps_ms = psms.tile([1, RCH], F32, tag="ms")
            for i in range(ND // 2):
                nc.tensor.matmul(ps_ms, ones8, sq[:, 2 * i:2 * i + 2, :],
                                 start=(i == 0), stop=(i == ND // 2 - 1),
                                 perf_mode=DR)
            sr = st.tile([1, RCH], F32, tag="sr")
            nc.scalar.activation(sr, ps_ms,
                                 mybir.ActivationFunctionType.Sqrt,
                                 bias=eps_sb, scale=1.0 / D)
            rf = st.tile([1, RCH], F32, tag="rf")
            nc.vector.reciprocal(rf, sr)
            rb = st.tile([1, RCH], BF16, tag="rb")
            nc.vector.tensor_copy(rb, rf)
            # broadcast r across partitions via a K=1 ones-matmul
            psr = psms.tile([P, RCH], F32, tag="psr")
            nc.tensor.matmul(psr, onesc, rb, start=True, stop=True)
            rep = p12.tile([P, RCH], BF16, tag="rep")
            nc.scalar.copy(rep, psr)
            # h^T = x^T * r (broadcast over d), quantized to fp8
            hT = p12.tile([P, ND, RCH], F8, tag="hT")
            for di in range(ND):
                nc.vector.tensor_tensor(hT[:, di, :], xT[:, di, :], rep,
                                        mybir.AluOpType.mult)

            cos_c = p12.tile([P, 4, RCH], BF16, tag="cos")
            nc.sync.dma_start(cos_c, cos_r[:, :, rows])
            sin_c = p12.tile([P, 4, RCH], BF16, tag="sin")
            nc.sync.dma_start(sin_c, sin_r[:, :, rows])

            # Q^T (chunks 0..3) and K^T: psum -> bf16 -> rope -> fp8 resident
            sels = [("k", D, kt_s)] + ([("q", 0, qt_s)] if r < N_QCH else [])
            for _, base, dst in sels:
                t_qk = p12.tile([P, ND, RCH], BF16, tag="tqk")
                for do in range(ND):
                    ps = ps12.tile([P, RCH], F32, tag="ps12")
                    for i in range(ND // 2):
                        nc.tensor.matmul(
                            ps,
                            wqkv_sb[:, 2 * i:2 * i + 2,
                                    base + do * P: base + (do + 1) * P],
                            hT[:, 2 * i:2 * i + 2, :],
                            start=(i == 0), stop=(i == ND // 2 - 1),
                            perf_mode=DR)
                    if has_bqkv:
                        nc.scalar.activation(
                            t_qk[:, do, :], ps,
                            mybir.ActivationFunctionType.Identity,
                            scale=1.0 / WSCALE,
                            bias=bqkv_sb[:, base // P + do: base // P + do + 1])
                    else:
                        nc.scalar.activation(
                            t_qk[:, do, :], ps,
                            mybir.ActivationFunctionType.Copy,
                            scale=1.0 / WSCALE)
                # rope: pair block pr with pr+4, write fp8 into resident dst
                for pr in range(4):
                    m1 = tmp12.tile([P, RCH], BF16, tag="m1")
                    nc.vector.tensor_tensor(m1, t_qk[:, pr, :],
                                            cos_c[:, pr, :],
                                            mybir.AluOpType.mult)
                    m2 = tmp12.tile([P, RCH], BF16, tag="m2")
                    nc.vector.tensor_tensor(m2, t_qk[:, pr + 4, :],
                                            sin_c[:, pr, :],
                                            mybir.AluOpType.mult)
                    nc.vector.tensor_tensor(dst[:, pr, rows], m1, m2,
                                            mybir.AluOpType.subtract)
                    m3 = tmp12.tile([P, RCH], BF16, tag="m3")
                    nc.vector.tensor_tensor(m3, t_qk[:, pr + 4, :],
                                            cos_c[:, pr, :],
                                            mybir.AluOpType.mult)
                    m4 = tmp12.tile([P, RCH], BF16, tag="m4")
                    nc.vector.tensor_tensor(m4, t_qk[:, pr, :],
                                            sin_c[:, pr, :],
                                            mybir.AluOpType.mult)
                    nc.vector.tensor_tensor(dst[:, pr + 4, rows], m3, m4,
                                            mybir.AluOpType.add)

            # V (keys on partitions): lhsT = h^T row-block, rhs = Wv
            for sub in range(RCH // P):
                for no in range(D // 512):
                    ps = ps12.tile([P, RCH], F32, tag="ps12")
                    for i in range(ND // 2):
                        nc.tensor.matmul(
                            ps,
                            hT[:, 2 * i:2 * i + 2, sub * P:(sub + 1) * P],
                            wqkv_sb[:, 2 * i:2 * i + 2,
                                    2 * D + no * 512: 2 * D + (no + 1) * 512],
                            start=(i == 0), stop=(i == ND // 2 - 1),
                            perf_mode=DR)
                    vdst = v_s[:, r * (RCH // P) + sub, no * 512:(no + 1) * 512]
                    if has_bqkv:
                        vt = tmp12.tile([P, 512], BF16, tag="vtb")
                        nc.scalar.activation(vt, ps,
                                             mybir.ActivationFunctionType.Copy,
                                             scale=1.0 / WSCALE)
                        nc.vector.tensor_tensor(
                            vdst, vt,
                            bass.AP(tensor=bqkv_d.tensor,
                                    offset=bqkv_d.offset + 2 * D + no * 512,
                                    ap=[[0, P], [1, 512]]),
                            mybir.AluOpType.add)
                    else:
                        nc.scalar.activation(vdst, ps,
                                             mybir.ActivationFunctionType.Copy,
                                             scale=1.0 / WSCALE)


def _phase3(nc, tc, S, wp_sb, ident64, res_d, out_d, kt_s, qt_s, v_s):
    N_QCH = max((S // 2) // RCH, 1)
    NKT = S // P
    NSUB = RCH // P
    with (
        tc.tile_pool(name="p3", bufs=2) as p3,
        tc.tile_pool(name="outp", bufs=4) as outp,
        tc.tile_pool(name="rcp", bufs=4) as rcp,
        tc.tile_pool(name="ps_s", bufs=2, space="PSUM") as ps_s,
        tc.tile_pool(name="ps_pv", bufs=1, space="PSUM") as ps_pv,
        tc.tile_pool(name="ps_pj", bufs=2, space="PSUM") as ps_pj,
    ):
        for c in range(N_QCH):
            qcols = slice(c * RCH, (c + 1) * RCH)
            pt = p3.tile([P, NKT, RCH], F8, tag="pt")
            acc = p3.tile([P, RCH], F32, tag="acc")
            recip = rcp.tile([P, NSUB], F32, tag="recip")
            for kt in range(NKT):
                ps = ps_s.tile([P, RCH], F32, tag="ps_s")
                for i in range(ND // 2):
                    nc.tensor.matmul(ps,
                                     kt_s[:, 2 * i:2 * i + 2, kt * P:(kt + 1) * P],
                                     qt_s[:, 2 * i:2 * i + 2, qcols],
                                     start=(i == 0), stop=(i == ND // 2 - 1),
                                     perf_mode=DR)
                nc.scalar.activation(pt[:, kt, :], ps,
                                     mybir.ActivationFunctionType.Exp,
                                     scale=LN3 / 32.0)
                if kt == 0:
                    nc.vector.tensor_copy(acc, pt[:, 0, :])
                else:
                    nc.vector.tensor_tensor(acc, acc, pt[:, kt, :],
                                            mybir.AluOpType.add)
            # per-query softmax sum: transpose (pre-scaled by 1/64) + reduce
            for i in range(NSUB):
                pst = ps_s.tile([P, P], F32, tag="ps_s", name=f"pstr{c}_{i}")
                nc.tensor.transpose(pst, acc[:, i * P:(i + 1) * P], ident64)
                scol = rcp.tile([P, 1], F32, tag="scol")
                nc.vector.reduce_sum(scol, pst, axis=mybir.AxisListType.X)
                nc.vector.reciprocal(recip[:, i:i + 1], scol)

            # attn @ V, unnormalized, scaled by 1/64 into fp8
            ot = p3.tile([P, ND, RCH], F8, tag="ot")
            for g in range(2):
                pvs = [ps_pv.tile([P, RCH], F32, tag=f"pv{j}",
                                  name=f"pv{c}_{g}_{j}")
                       for j in range(4)]
                for t in range(NKT // 2):
                    for j in range(4):
                        nc.tensor.matmul(
                            pvs[j],
                            v_s[:, 2 * t:2 * t + 2,
                                g * 512 + j * P: g * 512 + (j + 1) * P],
                            pt[:, 2 * t:2 * t + 2, :],
                            start=(t == 0), stop=(t == NKT // 2 - 1),
                            perf_mode=DR)
                for j in range(4):
                    nc.scalar.activation(ot[:, g * 4 + j, :], pvs[j],
                                         mybir.ActivationFunctionType.Copy,
                                         scale=OSCALE)

            # out = (ot @ wp) * (64/sum) + res
            for qs in range(NSUB):
                for no in range(D // 512):
                    ps = ps_pj.tile([P, 512], F32, tag="pj")
                    for i in range(ND // 2):
                        nc.tensor.matmul(
                            ps, ot[:, 2 * i:2 * i + 2, qs * P:(qs + 1) * P],
                            wp_sb[:, 2 * i:2 * i + 2, no * 512:(no + 1) * 512],
                            start=(i == 0), stop=(i == ND // 2 - 1),
                            perf_mode=DR)
                    o1 = outp.tile([P, 512], F32, tag="o1")
                    nc.vector.tensor_scalar_mul(o1, ps, recip[:, qs:qs + 1])
                    rt = outp.tile([P, 512], F32, tag="rt")
                    row0 = c * RCH + qs * P
                    nc.sync.dma_start(
                        rt, res_d[row0:row0 + P, no * 512:(no + 1) * 512])
                    o2 = outp.tile([P, 512], F32, tag="o2")
                    nc.vector.tensor_tensor(o2, o1, rt,
                                            mybir.AluOpType.add)
                    nc.sync.dma_start(
                        out_d[row0:row0 + P, no * 512:(no + 1) * 512], o2)


_CACHED = {}


def kernel(x, g_norm, w_qkv, b_qkv, w_proj, b_proj):
    global LAST_RESULT
    x = np.asarray(x, dtype=np.float32)
    g_norm = np.asarray(g_norm, dtype=np.float32)
    w_qkv = np.asarray(w_qkv, dtype=np.float32)
    b_qkv = np.asarray(b_qkv, dtype=np.float32)
    w_proj = np.asarray(w_proj, dtype=np.float32)
    b_proj = np.asarray(b_proj, dtype=np.float32)

    has_bqkv = bool(np.any(b_qkv))
    key = ("nc", has_bqkv)
    if key not in _CACHED:
        _CACHED[key] = _build(has_bqkv)
    nc = _CACHED[key]

    in_maps = _prepare_in_maps(x, g_norm, w_qkv, b_qkv, w_proj, b_proj)
    LAST_RESULT = run_bass_kernel_spmd(nc, in_maps, list(range(N_CORES)),
                                       trace=False)
    out = np.empty((B, S, D), dtype=np.float32)
    for c in range(N_CORES):
        b, h = c // 2, c % 2
        out[b, h * HALF:(h + 1) * HALF, :] = LAST_RESULT.results[c]["out"]
    return out
